# revision 7
# baseline (speedup 1.0000x reference)
"""Fused multi-head attention (QKV + RoPE2D + softmax + out-proj) on 8 TRN2 cores.

Sharding: batch-parallel. B == n_cores == 8, so each core runs one batch
element end-to-end; weights are replicated. No collectives needed.

v2 restructure vs baseline (322us):
  - eager pair-0 QKV+RoPE before the v projection, so the exp chain
    starts at ~13us instead of ~68us; v tiles produced JIT via feeder
  - all matmuls K=128/(128,128) tile mode: scores lhsT is a zero-padded
    per-head [128,N] k tile, normalization lhsT a padded [128,128]
    selection matrix -> no PE tiling-mode switches anywhere
  - per-PAIR softmax normalization (K=2-in-128 selection matmul) instead
    of per-8-head batches, so output projection ct-columns unlock early
  - out-proj split 4+3+1: ct0-3 chains after pair-3 norm, ct4-6 after
    pair-6 norm (both via feeder, PSUM->SBUF yacc), ct7 at the tail
  - sums row evacuated by ScalarE (ACT) instead of DVE; even head lands
    directly in the pair sums tile, odd head bounces via SBUF->SBUF DMA
  - feeder work is issued BETWEEN exp and AV so the in-order PE queue
    fills the exp window instead of stalling on AV
"""

import os
from collections import deque
import numpy as np

B, N, C = 8, 1024, 1024
H, HD = 16, 64
P = 128
NT = N // P          # 8 n-tiles
CT = C // P          # 8 c-tiles
TP = H // 2          # 8 head-pairs
SCALE = HD ** -0.5   # 1/8

KDTYPE = os.environ.get("BASS_ATTN_DTYPE", "bfloat16")
# feeder items are CHAIN-atomic (a full PSUM accumulation chain per yield,
# so an appendleft-preempting generator can never interleave into an open
# accumulation group in the shared mi_ps ring); ~2 chains fill an exp window
PULL_K = int(os.environ.get("BASS_ATTN_PULL", "2"))

_CACHE = {}


def _build_nc():
    import concourse.mybir as mybir
    from concourse import bacc, tile
    from contextlib import ExitStack

    f32 = mybir.dt.float32
    mdt = getattr(mybir.dt, KDTYPE)

    nc = bacc.Bacc(
        "TRN2", target_bir_lowering=False, debug=False,
        enable_asserts=False, num_devices=B,
    )

    xT_d = nc.dram_tensor("xT", [C, N], mdt, kind="ExternalInput")
    cos2_d = nc.dram_tensor("cos2", [P, N], f32, kind="ExternalInput")
    sin2_d = nc.dram_tensor("sin2", [P, N], f32, kind="ExternalInput")
    perm_d = nc.dram_tensor("perm", [P, P], mdt, kind="ExternalInput")
    wq_d = nc.dram_tensor("wq", [TP, CT, P, P], mdt, kind="ExternalInput")
    wk_d = nc.dram_tensor("wk", [TP, CT, P, P], mdt, kind="ExternalInput")
    wv_d = nc.dram_tensor("wv", [CT, P, C], mdt, kind="ExternalInput")
    wp_d = nc.dram_tensor("wp", [CT, P, C], mdt, kind="ExternalInput")
    sel2_d = nc.dram_tensor("sel2", [P, P], mdt, kind="ExternalInput")
    bias_d = nc.dram_tensor("bias", [1, C], f32, kind="ExternalInput")
    out_d = nc.dram_tensor("out", [N, C], f32, kind="ExternalOutput")

    EXP = mybir.ActivationFunctionType.Exp

    with tile.TileContext(nc) as tc, ExitStack() as ctx:
        const = ctx.enter_context(tc.tile_pool(name="const", bufs=1))
        vpool = ctx.enter_context(tc.tile_pool(name="vpool", bufs=1))
        otpool = ctx.enter_context(tc.tile_pool(name="otpool", bufs=1))
        qkpre = ctx.enter_context(tc.tile_pool(name="qkpre", bufs=2))
        tmpp = ctx.enter_context(tc.tile_pool(name="tmpp", bufs=2))
        expp = ctx.enter_context(tc.tile_pool(name="expp", bufs=3))
        wch = ctx.enter_context(tc.tile_pool(name="wch", bufs=3))
        ypool = ctx.enter_context(tc.tile_pool(name="ypool", bufs=4))
        sT_ps = ctx.enter_context(tc.tile_pool(name="sT_ps", bufs=2, space="PSUM"))
        av_ps = ctx.enter_context(tc.tile_pool(name="av_ps", bufs=1, space="PSUM"))
        mi_ps = ctx.enter_context(tc.tile_pool(name="mi_ps", bufs=2, space="PSUM"))

        # ---- input DMA, priority order ----
        xt = const.tile([P, CT, N], mdt)
        for ct in range(CT):
            nc.sync.dma_start(xt[:, ct, 0:512], xT_d[ct * P:(ct + 1) * P, 0:512])
        perm = const.tile([P, P], mdt)
        nc.sync.dma_start(perm[:], perm_d[:])
        cos2 = const.tile([P, N], f32)
        nc.sync.dma_start(cos2[:], cos2_d[:])
        sin2 = const.tile([P, N], f32)
        nc.sync.dma_start(sin2[:], sin2_d[:])
        for ct in range(CT):
            nc.sync.dma_start(xt[:, ct, 512:N], xT_d[ct * P:(ct + 1) * P, 512:N])
        sel2c = const.tile([P, P], mdt)
        nc.sync.dma_start(sel2c[:], sel2_d[:])
        bias_bc = const.tile([P, C], f32)
        nc.sync.dma_start(bias_bc[:1, :], bias_d[:])

        # warm the ACT exp table during the DMA wait
        scratch1 = const.tile([1, 16], f32)
        nc.vector.memset(scratch1[:], 0.0)
        nc.scalar.activation(scratch1[:], scratch1[:], EXP, scale=1.0)

        # zero-padded per-head k tiles (rows of the other head stay 0),
        # double-buffered by pair parity
        kpad = [[const.tile([P, N], mdt, name=f"kpad{p}{hh}") for hh in range(2)]
                for p in range(2)]
        for p in range(2):
            for hh in range(2):
                nc.vector.memset(kpad[p][hh][:], 0.0)
        finq = [const.tile([P, N], mdt, name=f"finq{p}") for p in range(2)]
        # padded normalization rhs: rows 0-1 hold the pair reciprocals,
        # rows 2-127 must stay zero (sel2 has zero weights there, but
        # garbage NaN * 0 would poison the matmul)
        rcp16 = const.tile([P, N], mdt)
        nc.vector.memset(rcp16[:], 0.0)
        rcpf = const.tile([2, N], f32)
        spair = [const.tile([2, N], f32, name=f"spair{p}") for p in range(2)]
        cpb = [const.tile([1, N], f32, name=f"cpb{p}") for p in range(2)]

        nc.gpsimd.partition_broadcast(bias_bc[:], bias_bc[:1, :])

        # v storage: [128 j-local, NT j-tiles, H heads x (64 v + 1 ones col)]
        v_all = vpool.tile([P, NT, H * (HD + 1)], mdt)
        ones_c = const.tile([P, H], f32)
        nc.vector.memset(ones_c[:], 1.0)
        for jt in range(NT):
            nc.vector.tensor_copy(
                v_all[:, jt, :].rearrange("p (h c) -> p h c", c=HD + 1)[:, :, HD:],
                ones_c[:].rearrange("p (h o) -> p h o", o=1))
        # out.T accumulator [128 c-local, CT pair-tiles, N] and the
        # out-proj SBUF accumulator [128 n-local, NT, C]
        outT = otpool.tile([P, CT, N], mdt)
        yacc = otpool.tile([P, NT, C], f32)

        wv = const.tile([P, CT, C], mdt)
        wp = const.tile([P, CT, C], mdt)

        feed = deque()

        def pull(k):
            n = 0
            while feed and n < k:
                if next(feed[0], "done") == "done":
                    feed.popleft()
                else:
                    n += 1

        def drain(g):
            for _ in g:
                pass

        def gen_qk(t):
            """QKV D-tiles + RoPE for pair t. q side -> finq[par] (full
            [128,N]); k side -> kpad[par][hh] zero-padded per head."""
            par = t % 2
            for wsrc, side in ((wq_d, "q"), (wk_d, "k")):
                wcht = wch.tile([P, CT, P], mdt, tag="w", name="wcht")
                nc.sync.dma_start(wcht[:], wsrc[t].rearrange("a p c -> p a c"))
                pre = qkpre.tile([P, N], mdt, tag="pre", name="pre")
                for ch in range(2):
                    sl = slice(ch * 512, (ch + 1) * 512)
                    qps = mi_ps.tile([P, 512], f32, tag="mi", name="qps")
                    for ct in range(CT):
                        nc.tensor.matmul(
                            qps[:], wcht[:, ct, :], xt[:, ct, sl],
                            start=(ct == 0), stop=(ct == CT - 1))
                    nc.scalar.copy(pre[:, sl], qps[:])
                    yield
                for ch in range(2):
                    sl = slice(ch * 512, (ch + 1) * 512)
                    rot = mi_ps.tile([P, 512], f32, tag="mi", name="rot")
                    nc.tensor.matmul(rot[:], perm[:], pre[:, sl],
                                     start=True, stop=True)
                    yield
                    tmp = tmpp.tile([P, 512], f32, tag="tmp", name="tmp")
                    nc.vector.tensor_mul(tmp[:], pre[:, sl], cos2[:, sl])
                    if side == "q":
                        fin = finq[par]
                        nc.vector.tensor_mul(fin[:, sl], rot[:], sin2[:, sl])
                        nc.vector.tensor_add(fin[:, sl], fin[:, sl], tmp[:])
                    else:
                        for hh in range(2):
                            ro = slice(64 * hh, 64 * hh + 64)
                            kp = kpad[par][hh]
                            nc.vector.tensor_mul(kp[ro, sl], rot[ro, :],
                                                 sin2[ro, sl])
                            nc.vector.tensor_add(kp[ro, sl], kp[ro, sl],
                                                 tmp[ro, :])

        def gen_v(nt, ch):
            vps = mi_ps.tile([P, 512], f32, tag="mi", name="vps")
            for ct in range(CT):
                nc.tensor.matmul(
                    vps[:], xt[:, ct, nt * P:(nt + 1) * P],
                    wv[:, ct, ch * 512:(ch + 1) * 512],
                    start=(ct == 0), stop=(ct == CT - 1))
            yield
            nc.vector.tensor_copy(
                v_all[:, nt, :].rearrange(
                    "p (h c) -> p h c", c=HD + 1)[:, 8 * ch:8 * ch + 8, :HD],
                vps[:])

        def gen_proj(nt, ch, cts, first):
            sl = slice(ch * 512, (ch + 1) * 512)
            pps = mi_ps.tile([P, 512], f32, tag="mi", name="pps")
            for i, ct in enumerate(cts):
                nc.tensor.matmul(
                    pps[:], outT[:, ct, nt * P:(nt + 1) * P], wp[:, ct, sl],
                    start=(i == 0), stop=(i == len(cts) - 1))
            yield
            if first:
                nc.vector.tensor_add(yacc[:, nt, sl], pps[:], bias_bc[:, sl])
            else:
                nc.vector.tensor_add(yacc[:, nt, sl], yacc[:, nt, sl], pps[:])

        def norm_pair(t):
            par = t % 2
            nc.vector.reciprocal_approx_fast(rcpf[:], spair[par][:])
            nc.vector.tensor_copy(rcp16[0:2, :], rcpf[:])
            for ch in range(2):
                sl = slice(ch * 512, (ch + 1) * 512)
                rb = mi_ps.tile([P, 512], f32, tag="mi", name="rb")
                nc.tensor.matmul(rb[:], sel2c[:], rcp16[:, sl],
                                 start=True, stop=True)
                yield
                nc.vector.tensor_mul(outT[:, t, sl], outT[:, t, sl], rb[:])

        def gen_norm(t):
            yield
            yield
            yield
            yield from norm_pair(t)
            if t == 3:
                for nt in range(NT):
                    for ch in range(2):
                        feed.append(gen_proj(nt, ch, range(0, 4), True))
            if t == 6:
                for nt in range(NT):
                    for ch in range(2):
                        feed.append(gen_proj(nt, ch, range(4, 7), False))

        # ---- eager pair 0 prep + first v tile ----
        drain(gen_qk(0))
        for ct in range(CT):
            nc.sync.dma_start(wv[:, ct, 0:512], wv_d[ct][:, 0:512])
        for ct in range(CT):
            nc.sync.dma_start(wv[:, ct, 512:C], wv_d[ct][:, 512:C])
        for ct in range(CT):
            nc.sync.dma_start(wp[:, ct, :], wp_d[ct])
        drain(gen_v(0, 0))

        # ---- attention: serial heads, feeder-filled exp windows ----
        for h in range(H):
            t, hh = h // 2, h % 2
            par = t % 2
            ro = slice(64 * hh, 64 * hh + 64)
            if h == 1:
                feed.append(gen_qk(1))
            elif h == 3:
                feed.append(gen_qk(2))
                for nt in range(NT):
                    feed.append(gen_v(nt, 1))
            elif hh == 0 and 2 <= t < TP - 1:
                feed.append(gen_qk(t + 1))

            av = av_ps.tile([HD + 1, N], f32, tag="av", name="av")
            for jt in range(NT):
                sT = sT_ps.tile([P, N], f32, tag="sT", name="sT")
                for ch in range(2):
                    sl = slice(ch * 512, (ch + 1) * 512)
                    nc.tensor.matmul(
                        sT[:, sl], kpad[par][hh][:, jt * P:(jt + 1) * P],
                        finq[par][:, sl], start=True, stop=True)
                ex = expp.tile([P, N], mdt, tag="ex", name="ex")
                nc.scalar.activation(ex[:], sT[:], EXP, scale=SCALE)
                if h == 0:
                    if jt < NT - 1:
                        drain(gen_v(jt + 1, 0))
                else:
                    pull(PULL_K)
                vh = v_all[:, jt, h * (HD + 1):(h + 1) * (HD + 1)]
                for ch in range(2):
                    sl = slice(ch * 512, (ch + 1) * 512)
                    nc.tensor.matmul(
                        av[:, sl], vh, ex[:, sl],
                        start=(jt == 0), stop=(jt == NT - 1))
            # head end: evacuate out rows (DVE) + sums row (ACT)
            nc.vector.tensor_copy(outT[ro, t, :], av[:HD, :])
            if hh == 0:
                nc.scalar.copy(spair[par][0:1, :], av[HD:HD + 1, :])
            else:
                nc.scalar.copy(cpb[par][:], av[HD:HD + 1, :])
                nc.sync.dma_start(spair[par][1:2, :], cpb[par][:])
                if t < TP - 1:
                    feed.appendleft(gen_norm(t))

        # ---- tail: drain feeder, final pair norm, proj ct7, DMA out ----
        pull(10 ** 6)
        drain(norm_pair(TP - 1))
        for nt in range(NT):
            for ch in range(2):
                sl = slice(ch * 512, (ch + 1) * 512)
                pps = mi_ps.tile([P, 512], f32, tag="mi", name="ppsz")
                nc.tensor.matmul(pps[:], outT[:, CT - 1, nt * P:(nt + 1) * P],
                                 wp[:, CT - 1, sl], start=True, stop=True)
                yb = ypool.tile([P, 512], f32, tag="yb", name="yb")
                nc.vector.tensor_add(yb[:], pps[:], yacc[:, nt, sl])
                nc.sync.dma_start(out_d[nt * P:(nt + 1) * P, sl], yb[:])

    nc.compile()
    return nc


def get_nc():
    if "nc" not in _CACHE:
        _CACHE["nc"] = _build_nc()
    return _CACHE["nc"]


def _host_inputs(x, xpos, w_qkv, w_proj, b_proj):
    """Host-side reshapes: transposes, RoPE tables, weight packing."""
    x = np.asarray(x, dtype=np.float32)
    xpos = np.asarray(xpos)
    w_qkv = np.asarray(w_qkv, dtype=np.float32)
    w_proj = np.asarray(w_proj, dtype=np.float32)
    b_proj = np.asarray(b_proj, dtype=np.float32).reshape(1, C)

    xT = np.ascontiguousarray(x.transpose(0, 2, 1))  # [B, C, N]

    # RoPE tables in [d, n] orientation, two head-copies stacked to 128 rows.
    inv_freq = (100.0 ** (-np.arange(16, dtype=np.float64) / 16.0))
    py = xpos[..., 0].astype(np.float64)  # [B, N]
    px = xpos[..., 1].astype(np.float64)
    angy = py[:, :, None] * inv_freq      # [B, N, 16]
    angx = px[:, :, None] * inv_freq
    cos64 = np.concatenate(
        [np.cos(angy), np.cos(angy), np.cos(angx), np.cos(angx)], axis=2)
    sin64 = np.concatenate(
        [-np.sin(angy), np.sin(angy), -np.sin(angx), np.sin(angx)], axis=2)
    cos2 = np.ascontiguousarray(
        np.tile(cos64, (1, 1, 2)).transpose(0, 2, 1)).astype(np.float32)
    sin2 = np.ascontiguousarray(
        np.tile(sin64, (1, 1, 2)).transpose(0, 2, 1)).astype(np.float32)

    # permutation matrix: sigma(d) = d XOR 16 within each 64-block
    r = np.arange(P)
    sig = (r // 64) * 64 + ((r % 64) ^ 16)
    perm = np.zeros((P, P), dtype=np.float32)
    perm[sig, r] = 1.0  # perm[k, m] = 1 iff k == sigma(m)

    wq = np.zeros((TP, CT, P, P), dtype=np.float32)
    wk = np.zeros((TP, CT, P, P), dtype=np.float32)
    for t in range(TP):
        for ct in range(CT):
            wq[t, ct] = w_qkv[t * P:(t + 1) * P, ct * P:(ct + 1) * P].T
            wk[t, ct] = w_qkv[C + t * P:C + (t + 1) * P, ct * P:(ct + 1) * P].T
    wv = np.ascontiguousarray(
        w_qkv[2 * C:3 * C, :].T.reshape(CT, P, C))   # [ct][c-local, dd]
    wp = np.ascontiguousarray(w_proj.T.reshape(CT, P, C))  # [ct][c-local, e]

    # padded pair-selection matrix: row 0 -> out cols 0-63, row 1 -> 64-127
    sel2 = np.zeros((P, P), dtype=np.float32)
    sel2[0, :HD] = 1.0
    sel2[1, HD:] = 1.0

    if KDTYPE == "bfloat16":
        import ml_dtypes

        def mcast(a):
            return np.ascontiguousarray(a).astype(ml_dtypes.bfloat16)
    else:
        def mcast(a):
            return np.ascontiguousarray(a)

    shared = dict(perm=mcast(perm), wq=mcast(wq), wk=mcast(wk),
                  wv=mcast(wv), wp=mcast(wp), sel2=mcast(sel2), bias=b_proj)
    in_maps = []
    for b in range(B):
        m = dict(shared)
        m["xT"] = mcast(xT[b])
        m["cos2"] = cos2[b]
        m["sin2"] = sin2[b]
        in_maps.append(m)
    return in_maps


def kernel(x, xpos, w_qkv, w_proj, b_proj):
    from concourse import bass_utils

    nc = get_nc()
    in_maps = _host_inputs(x, xpos, w_qkv, w_proj, b_proj)
    res = bass_utils.run_bass_kernel_spmd(
        nc, in_maps, core_ids=list(range(B)),
        trace=bool(int(os.environ.get("BASS_ATTN_TRACE", "0"))),
    )
    out = np.stack([res.results[b]["out"] for b in range(B)], axis=0)
    _CACHE["last_results"] = res
    return out
